# revision 34
# baseline (speedup 1.0000x reference)
"""Trainium2 Bass kernel for a fused multi-head attention layer.

Math (per batch b):
    xh = x.reshape(S, H, d); q/k/v = xh @ W{q,k,v}[h] + b
    scores = q @ k^T  (per head);  scores[-1, -1024:] = -inf
    attn = softmax(scores, -1) / sqrt(D)
    o = concat_h(attn @ v);  proj = o @ Wo + bo
    out = LayerNorm(x + proj) * g + beta

Sharding: 8 cores = 2 batches x 4 query-blocks of 512 rows. Each core
computes K/V for its full batch (duplicated across the 4 cores of a
batch; ~9% extra flops) and Q/attention/projection/LN for its own 512
query rows. No collectives.

On-chip layout is "transposed": qT/kT are [d_model_rows, seq] so the
score matmuls contract over the head dim, producing scoresT [t, s]
tiles. exp() runs on ScalarE; the softmax denominator comes for free by
augmenting V with a ones-column inside the PV matmul (row 64 of the
attention-output PSUM tile = sum_t exp). The 1/denominator is broadcast
across partitions with a K=1 matmul and folded together with the
1/sqrt(D) post-softmax scale.
"""

import numpy as np
import ml_dtypes

import concourse.bass as bass
import concourse.mybir as mybir
import concourse.tile as tile
from concourse import bacc
from concourse.bass import ds, ts
from concourse.bass_utils import run_bass_kernel_spmd

BF16 = mybir.dt.bfloat16
F32 = mybir.dt.float32
AF = mybir.ActivationFunctionType
OP = mybir.AluOpType

B, S, D, H = 2, 2048, 1024, 16
d = 64            # head dim
NP = H // 2       # 8 head pairs
SQ = S // 4       # 512 query rows per core
TCK = S // 128    # 16 key chunks of 128
SEQ_LEN = 1024
SCALE = float(np.sqrt(D))
LN_EPS = 1e-5
N_CORES = 8


def _bcast(ap, p=128):
    """AP replicating `ap` across p partitions (partition step 0)."""
    return bass.AP(tensor=ap.tensor, offset=ap.offset, ap=[[0, p]] + list(ap.ap))


def build_nc(debug=False):
    nc = bacc.Bacc("TRN2")

    xT = nc.dram_tensor("xT", [D, S], BF16, kind="ExternalInput")       # x[b].T
    xqT = nc.dram_tensor("xqT", [D, SQ], BF16, kind="ExternalInput")    # x[b,rows].T
    xq = nc.dram_tensor("xq", [SQ, D], F32, kind="ExternalInput")       # x[b,rows]+bo
    wq = nc.dram_tensor("wq", [NP, 128, 128], BF16, kind="ExternalInput")
    wk = nc.dram_tensor("wk", [NP, 128, 128], BF16, kind="ExternalInput")
    wv = nc.dram_tensor("wv", [NP, 128, 128], BF16, kind="ExternalInput")
    bqk = nc.dram_tensor("bqk", [128, 2 * NP], F32, kind="ExternalInput")
    bvt = nc.dram_tensor("bvt", [NP, 128], F32, kind="ExternalInput")
    wo = nc.dram_tensor("wo", [D, D], BF16, kind="ExternalInput")
    gg = nc.dram_tensor("gg", [D], F32, kind="ExternalInput")
    bb = nc.dram_tensor("bb", [D], F32, kind="ExternalInput")
    msk = nc.dram_tensor("msk", [1, 1], F32, kind="ExternalInput")      # 1.0 / 0.0
    out = nc.dram_tensor("out", [SQ, D], F32, kind="ExternalOutput")
    dbg = {}
    if debug:
        dbg["kT0"] = nc.dram_tensor("d_kT0", [128, S], F32, kind="ExternalOutput")
        dbg["qT0"] = nc.dram_tensor("d_qT0", [128, SQ], F32, kind="ExternalOutput")
        dbg["ex0"] = nc.dram_tensor("d_ex0", [128, 2, 512], F32, kind="ExternalOutput")
        dbg["oA0"] = nc.dram_tensor("d_oA0", [128, 512], F32, kind="ExternalOutput")
        dbg["rc0"] = nc.dram_tensor("d_rc0", [128, 512], F32, kind="ExternalOutput")
        dbg["oT"] = nc.dram_tensor("d_oT", [128, NP, SQ], F32, kind="ExternalOutput")
        dbg["y0"] = nc.dram_tensor("d_y0", [128, D], F32, kind="ExternalOutput")

    with tile.TileContext(nc) as tc:
        with (
            tc.tile_pool(name="singles", bufs=1) as singles,
            tc.tile_pool(name="xpool", bufs=3) as xpool,
            tc.tile_pool(name="kpool", bufs=4) as kpool,
            tc.tile_pool(name="qpool", bufs=4) as qpool,
            tc.tile_pool(name="qxpool", bufs=2) as qxpool,
            tc.tile_pool(name="vpool", bufs=4) as vpool,
            tc.tile_pool(name="epool", bufs=4) as epool,
            tc.tile_pool(name="rpool", bufs=2) as rpool,
            tc.tile_pool(name="ypool", bufs=2) as ypool,
            tc.tile_pool(name="stpool", bufs=4) as stpool,
            tc.tile_pool(name="psA", bufs=2, space="PSUM") as psA,
            tc.tile_pool(name="psB", bufs=2, space="PSUM") as psB,
            tc.tile_pool(name="psD", bufs=2, space="PSUM") as psD,
        ):
            # ---- constants / weights ----
            wq_sb = singles.tile([128, NP, 128], BF16)
            wk_sb = singles.tile([128, NP, 128], BF16)
            wv_sb = singles.tile([128, NP, 128], BF16)
            nc.scalar.dma_start(out=wq_sb, in_=wq[:].rearrange("c p f -> p c f"))
            nc.scalar.dma_start(out=wk_sb, in_=wk[:].rearrange("c p f -> p c f"))
            nc.scalar.dma_start(out=wv_sb, in_=wv[:].rearrange("c p f -> p c f"))
            bqk_sb = singles.tile([128, 2 * NP], F32)
            nc.gpsimd.dma_start(out=bqk_sb, in_=bqk[:])
            bq_sb = bqk_sb[:, 0:NP]
            bk_sb = bqk_sb[:, NP:2 * NP]
            bv_bc = singles.tile([128, NP, 128], F32)
            nc.gpsimd.dma_start(out=bv_bc, in_=_bcast(bvt[:]))
            wo_sb = singles.tile([128, NP, D], BF16)
            nc.scalar.dma_start(out=wo_sb, in_=wo[:].rearrange("(c p) f -> p c f", p=128))
            xq_sb = singles.tile([128, 4, D], F32)
            nc.scalar.dma_start(out=xq_sb, in_=xq[:].rearrange("(m p) f -> p m f", p=128))
            g_bc = singles.tile([128, D], F32)
            b_bc = singles.tile([128, D], F32)
            nc.gpsimd.dma_start(out=g_bc, in_=_bcast(gg[:]))
            nc.gpsimd.dma_start(out=b_bc, in_=_bcast(bb[:]))
            msk_sb = singles.tile([128, 1], F32)
            nc.gpsimd.dma_start(out=msk_sb, in_=_bcast(msk[:].rearrange("a b -> (a b)")))
            eps_sb = singles.tile([128, 1], F32)
            nc.vector.memset(eps_sb, LN_EPS)
            ones_l = singles.tile([128, d], F32)
            nc.vector.memset(ones_l, 1.0 / SCALE)
            oT_sb = singles.tile([128, NP, SQ], BF16)

            # Touch every DMA-loaded constant once on VectorE so its vector
            # clock passes the DMA sems; later consumers then need no DMA
            # waits (walrus caps sync-waits per instruction).
            scr = singles.tile([128, 8], F32)
            for i, t in enumerate([bq_sb[:, 0:1], bk_sb[:, 0:1],
                                   bv_bc[:, 0, 0:1],
                                   g_bc[:, 0:1], b_bc[:, 0:1],
                                   msk_sb[:, 0:1]]):
                nc.vector.tensor_copy(out=scr[:, i:i + 1], in_=t)

            # ---- per head-pair: QKV projections, scores, softmax, PV ----
            # Software-pipelined emission: qkv(p+1) is emitted BEFORE
            # attention(p) so the scheduler prioritizes producing the next
            # pair's kT/qT/v during the current pair's attention stream --
            # otherwise ScalarE stalls ~8us at every pair boundary.
            qkv = {}

            def emit_qkv(p):
                xT_t = xpool.tile([128, S], BF16)
                for c4 in range(4):
                    nc.sync.dma_start(out=xT_t[:, ts(c4, 512)],
                                      in_=xT[ds(128 * p, 128), ts(c4, 512)])
                xqT_t = qxpool.tile([128, SQ], BF16)
                nc.sync.dma_start(out=xqT_t, in_=xqT[ds(128 * p, 128), :])

                # qT[e_pair, s] first (unblocks first scores), then kT
                qT_t = qpool.tile([128, SQ], BF16)
                ps = psD.tile([128, 512], F32, tag="qkv")
                nc.tensor.matmul(ps, lhsT=wq_sb[:, p, :], rhs=xqT_t,
                                 start=True, stop=True)
                nc.vector.tensor_scalar(out=qT_t, in0=ps,
                                        scalar1=bq_sb[:, p:p + 1], scalar2=None,
                                        op0=OP.add)
                kT_t = kpool.tile([128, S], BF16)
                for c in range(S // 512):
                    ps = psD.tile([128, 512], F32, tag="qkv")
                    nc.tensor.matmul(ps, lhsT=wk_sb[:, p, :], rhs=xT_t[:, ts(c, 512)],
                                     start=True, stop=True)
                    nc.vector.tensor_scalar(out=kT_t[:, ts(c, 512)], in0=ps,
                                            scalar1=bk_sb[:, p:p + 1], scalar2=None,
                                            op0=OP.add)

                # v[t, e] for both heads; layout [tc][hh][65]: cols 0:64 = v,
                # col 64 = 1.0 (PV ones-column -> softmax denominator row)
                v_t = vpool.tile([128, TCK, 2, 65], BF16)
                nc.gpsimd.memset(v_t[:, :, :, 64:65], 1.0)
                for tcx in range(TCK):
                    ps = psD.tile([128, 512], F32, tag="qkv")
                    nc.tensor.matmul(ps[:, 0:128], lhsT=xT_t[:, ds(128 * tcx, 128)],
                                     rhs=wv_sb[:, p, :], start=True, stop=True)
                    nc.vector.tensor_tensor(
                        out=v_t[:, tcx, :, 0:64],
                        in0=ps[:, 0:128].rearrange("a (h e) -> a h e", h=2),
                        in1=bv_bc[:, p, :].rearrange("a (h e) -> a h e", h=2),
                        op=OP.add)
                qkv[p] = (kT_t, qT_t, v_t)

            rcs = {}
            norm_pending = None

            def emit_norm_tail(pp):
                # broadcast 1/denom across the 64 e-rows per head (K=1
                # matmul, folds 1/sqrt(D)) and scale the stored oT block
                rc = rcs.pop(pp)
                bc = psA.tile([128, 512], F32, tag="sc")
                nc.tensor.matmul(bc[0:64, :], lhsT=ones_l[0:1, :],
                                 rhs=rc[0:1, 0, :], start=True, stop=True)
                nc.tensor.matmul(bc[64:128, :], lhsT=ones_l[0:1, :],
                                 rhs=rc[0:1, 1, :], start=True, stop=True)
                nc.vector.tensor_tensor(out=oT_sb[:, pp, :],
                                        in0=oT_sb[:, pp, :], in1=bc,
                                        op=OP.mult)

            emit_qkv(0)
            for p in range(NP):
                if p + 1 < NP:
                    emit_qkv(p + 1)
                kT_t, qT_t, v_t = qkv.pop(p)
                if debug and p == 0:
                    nc.gpsimd.dma_start(out=dbg["kT0"][:], in_=kT_t)
                    nc.gpsimd.dma_start(out=dbg["qT0"][:], in_=qT_t)

                # scoresT -> exp -> PV (accumulating over key chunks)
                oA = psB.tile([128, 512], F32, tag="ov")
                oB = psB.tile([128, 512], F32, tag="ov")
                exs = []
                for tcx in range(TCK):
                    sc = psA.tile([128, 2, 512], F32, tag="sc")
                    nc.tensor.matmul(sc[:, 0, :], lhsT=kT_t[0:64, ds(128 * tcx, 128)],
                                     rhs=qT_t[0:64, :], start=True, stop=True)
                    nc.tensor.matmul(sc[:, 1, :], lhsT=kT_t[64:128, ds(128 * tcx, 128)],
                                     rhs=qT_t[64:128, :], start=True, stop=True)
                    ex = epool.tile([128, 2, 512], BF16)
                    nc.scalar.activation(out=ex, in_=sc, func=AF.Exp)
                    if tcx >= TCK // 2:
                        # mask: query row 2047 (local col 511), keys >= 1024
                        nc.vector.tensor_scalar(
                            out=ex[:, :, 511:512], in0=ex[:, :, 511:512],
                            scalar1=msk_sb[:, 0:1], scalar2=None, op0=OP.mult)
                    if debug and p == 0 and tcx == 0:
                        nc.gpsimd.dma_start(out=dbg["ex0"][:], in_=ex)
                    exs.append(ex)
                    if tcx >= 1:
                        exl = exs[tcx - 1]
                        nc.tensor.matmul(oA[0:65, :], lhsT=v_t[:, tcx - 1, 0, :],
                                         rhs=exl[:, 0, :],
                                         start=(tcx == 1), stop=False)
                        nc.tensor.matmul(oB[0:65, :], lhsT=v_t[:, tcx - 1, 1, :],
                                         rhs=exl[:, 1, :],
                                         start=(tcx == 1), stop=False)
                    if tcx == 3 and norm_pending is not None:
                        emit_norm_tail(norm_pending)
                        norm_pending = None
                exl = exs[TCK - 1]
                nc.tensor.matmul(oA[0:65, :], lhsT=v_t[:, TCK - 1, 0, :],
                                 rhs=exl[:, 0, :], start=False, stop=True)
                nc.tensor.matmul(oB[0:65, :], lhsT=v_t[:, TCK - 1, 1, :],
                                 rhs=exl[:, 1, :], start=False, stop=True)

                # Drain oA/oB with DVE-only ops (so the PSUM slots free
                # without waiting on any PE work), compute 1/denominator.
                # The PE-side broadcast + final multiply (emit_norm_tail) is
                # deferred into the NEXT pair's attention stream so it never
                # blocks the in-order PE queue at the pair boundary.
                rs = rpool.tile([128, 2, 512], F32, tag="rs")
                rc = rpool.tile([128, 2, 512], F32)
                nc.vector.tensor_copy(out=rs[0:1, 0, :], in_=oA[64:65, :])
                nc.vector.tensor_copy(out=rs[0:1, 1, :], in_=oB[64:65, :])
                nc.vector.reciprocal_approx_fast(out=rc[0:1, :, :],
                                                 in_=rs[0:1, :, :])
                if debug and p == 0:
                    dt_ = rpool.tile([128, 512], F32, tag="dbg")
                    nc.vector.tensor_copy(out=dt_, in_=oA)
                    nc.gpsimd.dma_start(out=dbg["oA0"][:], in_=dt_)
                    nc.gpsimd.dma_start(out=dbg["rc0"][:], in_=rc[:, 0, :])
                nc.vector.tensor_copy(out=oT_sb[0:64, p, :], in_=oA[0:64, :])
                nc.vector.tensor_copy(out=oT_sb[64:128, p, :], in_=oB[0:64, :])
                rcs[p] = rc
                norm_pending = p
            emit_norm_tail(norm_pending)

            if debug:
                nc.gpsimd.dma_start(out=dbg["oT"][:], in_=oT_sb)

            # ---- output projection + residual + LayerNorm ----
            for m in range(4):
                y_t = ypool.tile([128, D], F32, tag="y")
                for fc in range(2):
                    pr = psA.tile([128, 2, 512], F32, tag="sc")
                    for p in range(NP):
                        nc.tensor.matmul(pr[:, 0, :], lhsT=oT_sb[:, p, ts(m, 128)],
                                         rhs=wo_sb[:, p, ts(fc, 512)],
                                         start=(p == 0), stop=(p == NP - 1))
                    nc.vector.tensor_tensor(out=y_t[:, ts(fc, 512)], in0=pr[:, 0, :],
                                            in1=xq_sb[:, m, ts(fc, 512)], op=OP.add)
                if debug and m == 0:
                    nc.gpsimd.dma_start(out=dbg["y0"][:], in_=y_t)
                st = stpool.tile([128, 2, 6], F32, tag="st")
                nc.vector.bn_stats(out=st[:, 0, :], in_=y_t[:, 0:512])
                nc.vector.bn_stats(out=st[:, 1, :], in_=y_t[:, 512:1024])
                mv = stpool.tile([128, 2], F32, tag="mv")
                nc.vector.bn_aggr(out=mv, in_=st)
                sd = stpool.tile([128, 1], F32, tag="sd")
                nc.scalar.activation(out=sd, in_=mv[:, 1:2], func=AF.Sqrt,
                                     bias=eps_sb[:, 0:1], scale=1.0)
                rstd = stpool.tile([128, 1], F32, tag="rs")
                nc.vector.reciprocal(out=rstd, in_=sd)
                yn = ypool.tile([128, D], F32, tag="yn")
                nc.vector.tensor_scalar(out=yn, in0=y_t, scalar1=mv[:, 0:1],
                                        scalar2=rstd, op0=OP.subtract, op1=OP.mult)
                ot = ypool.tile([128, D], F32, tag="ot")
                nc.vector.tensor_tensor(out=ot[:, 0:512], in0=yn[:, 0:512],
                                        in1=g_bc[:, 0:512], op=OP.mult)
                nc.gpsimd.tensor_tensor(out=ot[:, 512:1024], in0=yn[:, 512:1024],
                                        in1=g_bc[:, 512:1024], op=OP.mult)
                nc.vector.tensor_tensor(out=ot[:, 0:512], in0=ot[:, 0:512],
                                        in1=b_bc[:, 0:512], op=OP.add)
                nc.gpsimd.tensor_tensor(out=ot[:, 512:1024], in0=ot[:, 512:1024],
                                        in1=b_bc[:, 512:1024], op=OP.add)
                nc.sync.dma_start(out=out[ds(128 * m, 128), :], in_=ot)
    nc.compile()
    return nc


def prep_inputs(x, Wq, bq, Wk, bk, Wv, bv, Wo, bo, ln_g, ln_b):
    """Host-side sharding/layout prep -> list of 8 per-core input maps."""
    bf = ml_dtypes.bfloat16
    x = np.asarray(x, np.float32)
    Wq, Wk, Wv = (np.asarray(w, np.float32) for w in (Wq, Wk, Wv))
    Wo = np.asarray(Wo, np.float32)
    bq, bk, bv, bo = (np.asarray(v_, np.float32) for v_ in (bq, bk, bv, bo))
    ln_g, ln_b = np.asarray(ln_g, np.float32), np.asarray(ln_b, np.float32)

    def pairs(W):  # [H,d,d] -> [NP,128,128] block-diag
        out = np.zeros((NP, 128, 128), np.float32)
        for p in range(NP):
            out[p, :d, :d] = W[2 * p]
            out[p, d:, d:] = W[2 * p + 1]
        return out.astype(bf)

    wq_b, wk_b, wv_b = pairs(Wq), pairs(Wk), pairs(Wv)
    bqk = np.concatenate([bq.reshape(NP, 128).T, bk.reshape(NP, 128).T],
                         1).copy()             # [128, 2*NP]
    bvt = bv.reshape(NP, 128).copy()            # [NP, 128]
    wo_b = Wo.astype(bf)
    xT_all = [np.ascontiguousarray(x[b_].T).astype(bf) for b_ in range(B)]

    in_maps = []
    for c in range(N_CORES):
        b_, j = divmod(c, 4)
        rows = slice(j * SQ, (j + 1) * SQ)
        in_maps.append({
            "xT": xT_all[b_],
            "xqT": np.ascontiguousarray(xT_all[b_][:, rows]),
            "xq": (x[b_, rows] + bo).astype(np.float32),
            "wq": wq_b, "wk": wk_b, "wv": wv_b,
            "bqk": bqk, "bvt": bvt,
            "wo": wo_b,
            "gg": ln_g, "bb": ln_b,
            "msk": np.array([[0.0 if j == 3 else 1.0]], np.float32),
        })
    return in_maps


_NC = None


def _get_nc():
    global _NC
    if _NC is None:
        _NC = build_nc()
    return _NC


def _gather(results):
    y = np.empty((B, S, D), np.float32)
    for c, r in enumerate(results):
        b_, j = divmod(c, 4)
        y[b_, j * SQ:(j + 1) * SQ] = r["out"]
    return y


def kernel(**inputs):
    nc = _get_nc()
    in_maps = prep_inputs(**inputs)
    res = run_bass_kernel_spmd(nc, in_maps, core_ids=list(range(N_CORES)))
    return _gather(res.results)


def kernel_timed(**inputs):
    """Returns (output, exec_time_ns or None). Used by test.py."""
    nc = _get_nc()
    in_maps = prep_inputs(**inputs)
    res = run_bass_kernel_spmd(nc, in_maps, core_ids=list(range(N_CORES)),
                               trace=True)
    return _gather(res.results), res.exec_time_ns
